# revision 54
# baseline (speedup 1.0000x reference)
"""Multi-head attention (B=2, S=2048, D=1024, H=16, dk=dv=64) on 8 TRN2 cores.

Sharding: core c -> batch b = c % 2, head-group g = c // 2 (heads 4g..4g+3).
Each core computes its 4 heads' attention for one batch plus the partial
output projection; the host sums the 4 partials per batch and adds bo.

v2 design (vs the transpose-heavy v1): the whole input path is fp16
(halves HBM traffic; fp16's 11-bit mantissa keeps scores to ~1e-3), the
V projection is computed directly in natural [t, dv] layout (lhsT = the
V chunk itself), and every projection bias is a rank-1 K=1 matmul into
the accumulating PSUM so all PSUM->SBUF evictions are pure DVE copies.
The ACT engine then does nothing but the softmax exp, which is its hard
floor: (1024+352)/1.2GHz per [128,1024] tile, ~147us over the 128 tiles.
The attention pipeline runs one t-tile per step (scores -> exp -> ctx
trailing 2 steps) with a 3-deep scores-PSUM rotation so the PE can run
ahead of ACT and never bubbles long enough to re-throttle the HAM clock
gate. K/V/Q chunks stream in while block 0 is already computing; Q-proj,
out-proj and normalize work drains into later blocks' per-step slack.
"""
import os
import sys

sys.path.insert(0, "/opt/trn_rl_repo")
os.environ.setdefault("JAX_PLATFORMS", "axon,cpu")

from contextlib import ExitStack

import numpy as np

import concourse.bacc as bacc
import concourse.tile as tile
from concourse import mybir
from concourse.bass_utils import run_bass_kernel_spmd

FP16 = mybir.dt.float16
BF16 = mybir.dt.bfloat16
FP32 = mybir.dt.float32
FP32R = mybir.dt.float32r

B, S, D = 2, 2048, 1024
H, DK, DV = 16, 64, 64
N_CORES = 8
HPC = H // (N_CORES // B)  # heads per core = 4
P = 128
SBLK = 512                # s-block (free dim of scores matmuls)
NBLK = S // SBLK          # 4
NTT = S // P              # 16 t-tiles
NDC = D // P              # 8 contraction chunks
NV = HPC * (DV + 1)       # 260 (64 V cols + 1 denominator-ones col per head)
SCALE = 1.0 / (DK * 2.0)  # folded into Wv/bv
# brow packing offsets (one [1, 1284] fp16 row of constants)
ONES_OFF, BQ_OFF, BK_OFF, BVE_OFF = 0, 512, 768, 1024
BROW_W = 1284


def _build_nc():
    nc = bacc.Bacc("TRN2", target_bir_lowering=False, debug=False,
                   num_devices=N_CORES)
    # All bulk tensors are host-permuted so every DMA line is one long
    # contiguous row per partition (128 descriptors per transfer, not 1024
    # 1KB ones — descriptor generation was serializing the DMA queues).
    d = {}
    for name, shape, dt in [
        ("qt", [NBLK, P, NDC * SBLK], FP16), ("kt", [NBLK, P, NDC * SBLK], FP16),
        ("vt", [NBLK, P, NDC * SBLK], FP16),
        ("wq", [P, NDC * 2 * P], FP16), ("wk", [P, NDC * 2 * P], FP16),
        ("wv", [P, NDC * NV], FP16), ("wo", [P, 2 * D], FP16),
        ("brow", [1, BROW_W], FP16), ("onesdv", [1, DV], FP32),
    ]:
        d[name] = nc.dram_tensor(name, shape, dt, kind="ExternalInput").ap()
    out_d = nc.dram_tensor("out", [NBLK, P, 4 * D], FP16, kind="ExternalOutput").ap()

    with tile.TileContext(nc) as tc, ExitStack() as ctx:
        const = ctx.enter_context(tc.tile_pool(name="const", bufs=1))
        wpool = ctx.enter_context(tc.tile_pool(name="wpool", bufs=1))
        xtp = ctx.enter_context(tc.tile_pool(name="xtp", bufs=1))
        projp = ctx.enter_context(tc.tile_pool(name="projp", bufs=1))
        expp = ctx.enter_context(tc.tile_pool(name="expp", bufs=1))
        ctxp = ctx.enter_context(tc.tile_pool(name="ctxp", bufs=1))
        outp = ctx.enter_context(tc.tile_pool(name="outp", bufs=2))
        smallp = ctx.enter_context(tc.tile_pool(name="smallp", bufs=2))
        psum = ctx.enter_context(tc.tile_pool(name="psum", bufs=1, space="PSUM"))

        # ---- ACT table warm-up: a 2-elem exp triggers ACT_TABLE_LOAD
        # while the first DMAs are still in flight.
        dummy = smallp.tile([1, 2], FP32, tag="dmy")
        dummy2 = smallp.tile([1, 2], FP32, tag="dmy2")
        nc.vector.memset(dummy[:], 0.0)
        nc.scalar.activation(dummy2[:], dummy[:],
                             mybir.ActivationFunctionType.Exp)

        # ---- constants / weights (sync queue: K/Q path; gpsimd queue: V) ----
        brow = const.tile([1, BROW_W], FP16)
        nc.sync.dma_start(brow[:], d["brow"])
        wk_sb = wpool.tile([P, NDC, 2 * P], FP16)
        nc.sync.dma_start(wk_sb[:].rearrange("p a b -> p (a b)"), d["wk"])
        onesdv = const.tile([1, DV], FP32R)
        nc.gpsimd.dma_start(onesdv[:], d["onesdv"].bitcast(FP32R))
        wv_sb = wpool.tile([P, NDC, NV], FP16)
        nc.gpsimd.dma_start(wv_sb[:].rearrange("p a b -> p (a b)"), d["wv"])
        wq_sb = wpool.tile([P, NDC, 2 * P], FP16)
        wo_sb = wpool.tile([P, 2, D], FP16)

        # ---- persistent activation tiles ----
        kwt = [projp.tile([P, S], FP16, tag=f"kwt{p_}", name=f"kwt{p_}") for p_ in range(2)]
        qwt = [projp.tile([P, S], FP16, tag=f"qwt{p_}", name=f"qwt{p_}") for p_ in range(2)]
        vw = projp.tile([P, NTT, NV], BF16, tag="vw")
        ctx_t = [ctxp.tile([P, S], FP16, tag=f"ctx{p_}", name=f"ctx{p_}") for p_ in range(2)]

        def load_chunk(name, ci, eng=None):
            # kt/qt ride the sync DMA queue, vt the gpsimd queue: two
            # parallel streams halve the DMA-gated prologue.
            xt = xtp.tile([P, NDC, SBLK], FP16, tag="xt", name="xt", bufs=5)
            (eng or nc.sync).dma_start(
                xt[:].rearrange("p a b -> p (a b)"), d[name][ci])
            return xt

        def proj_qk_piece(xt, w_sb, dst, bias_off, ci, pair, dc_range, pq_holder):
            """Part of one head-pair x one 512-s-chunk projection; the final
            piece adds the rank-1 bias and DVE-evicts to fp16 SBUF."""
            if dc_range[0] == 0:
                pq_holder[pair] = psum.tile([P, 2, SBLK], FP32, tag="sc",
                                            name="pq", bufs=3)
            pq = pq_holder[pair]
            for dc in dc_range:
                nc.tensor.matmul(pq[:, 0, :], lhsT=w_sb[:, dc, pair * P:(pair + 1) * P],
                                 rhs=xt[:, dc, :], start=(dc == 0), stop=False)
            if dc_range[-1] == NDC - 1:
                nc.tensor.matmul(
                    pq[:, 0, :],
                    lhsT=brow[:, bias_off + pair * P:bias_off + (pair + 1) * P],
                    rhs=brow[:, ONES_OFF:ONES_OFF + SBLK],
                    start=False, stop=True)
                nc.vector.tensor_copy(dst[pair][:, ci * SBLK:(ci + 1) * SBLK],
                                      pq[:, 0, :])

        def proj_qk(xt, w_sb, dst, bias_off, ci, pair):
            h = [None, None]
            proj_qk_piece(xt, w_sb, dst, bias_off, ci, pair, range(NDC), h)

        def proj_v_piece(xt, ci, c, dc_range, pv_holder):
            """Part of one t-tile of the natural-layout V projection."""
            tt = ci * (SBLK // P) + c
            if dc_range[0] == 0:
                pv_holder[0] = psum.tile([P, 2, SBLK], FP32, tag="sc",
                                         name="pv", bufs=3)
            pv = pv_holder[0]
            for dc in dc_range:
                nc.tensor.matmul(pv[:, 0, 0:NV], lhsT=xt[:, dc, c * P:(c + 1) * P],
                                 rhs=wv_sb[:, dc, :], start=(dc == 0), stop=False)
            if dc_range[-1] == NDC - 1:
                nc.tensor.matmul(pv[:, 0, 0:NV], lhsT=brow[:, ONES_OFF:ONES_OFF + P],
                                 rhs=brow[:, BVE_OFF:BVE_OFF + NV],
                                 start=False, stop=True)
                nc.vector.tensor_copy(vw[:, tt, :], pv[:, 0, 0:NV])

        def proj_v_tt(xt, ci, c):
            h = [None]
            proj_v_piece(xt, ci, c, range(NDC), h)

        def attn_block(pair, b, fillers):
            """Per-t-tile pipeline: scores(k) -> exp(k) -> ctx(k-2).
            One 2-bank scores PSUM per step (hp0 | hp1), 3-deep rotation;
            exp is a single FD=1024 ACT instruction. `fillers` is a list of
            (slot, fn); fn is emitted when the step index reaches slot."""
            ct = [psum.tile([DV + 1, SBLK], FP32, tag=f"ct{hp}", name=f"ct{hp}")
                  for hp in range(2)]
            exs = {}
            for k in range(NTT + 2):
                if k < NTT:
                    sc = psum.tile([P, 2, SBLK], FP32, tag="sc", name="sc", bufs=3)
                    for hp in range(2):
                        lo, hi = hp * DK, (hp + 1) * DK
                        nc.tensor.matmul(
                            sc[:, hp, :],
                            lhsT=kwt[pair][lo:hi, k * P:(k + 1) * P],
                            rhs=qwt[pair][lo:hi, b * SBLK:(b + 1) * SBLK],
                            start=True, stop=True)
                    ex = expp.tile([P, 2, SBLK], BF16, tag="ex", name="ex", bufs=4)
                    nc.scalar.activation(ex[:], sc[:],
                                         mybir.ActivationFunctionType.Exp)
                    exs[k] = ex
                while fillers and fillers[0][0] <= k:
                    fillers.pop(0)[1]()
                kc = k - 2
                if kc >= 0:
                    ex = exs.pop(kc)
                    for hp in range(2):
                        hh = 2 * pair + hp
                        nc.tensor.matmul(
                            ct[hp][:], lhsT=vw[:, kc, hh * (DV + 1):(hh + 1) * (DV + 1)],
                            rhs=ex[:, hp, :],
                            start=(kc == 0), stop=(kc == NTT - 1))
            return ct

        def attn_normalize(pair, b, ct, hp):
            # ctx = ct[0:64] * (1 / ct[64]) row-broadcast; fp16 out
            den = smallp.tile([1, SBLK], FP32R, tag="den")
            nc.vector.tensor_copy(den[:], ct[hp][DV:DV + 1, :])
            rb = psum.tile([P, 2, SBLK], FP32, tag="sc", name="rb", bufs=3)
            nc.tensor.matmul(rb[0:DV, 0, :], lhsT=onesdv[:],
                             rhs=den[:], start=True, stop=True)
            rcp = smallp.tile([DV, SBLK], FP32, tag="rcp")
            nc.vector.reciprocal_approx_fast(rcp[:], rb[0:DV, 0, :])
            nc.vector.tensor_mul(
                ctx_t[pair][hp * DV:(hp + 1) * DV, b * SBLK:(b + 1) * SBLK],
                ct[hp][0:DV, :], rcp[:])

        ob_holder = [None]

        def out_proj_st(b, st):
            """One s-tile of the output projection: [128 s, 1024 D] via 4
            N=512 matmuls (fp16 moving operand caps at 512); the 4 s-tiles
            of a block stage into one SBUF tile DMA'd out as a single 1MB
            transfer on the gpsimd queue."""
            off = b * SBLK + st * P
            if st == 0:
                ob_holder[0] = outp.tile([P, 4, D], FP16, tag="ob", name="ob")
            po = psum.tile([P, 2, SBLK], FP32, tag="sc", name="po", bufs=3)
            for nh in range(2):
                for jc in range(2):
                    nc.tensor.matmul(po[:, nh, :],
                                     lhsT=ctx_t[jc][:, off:off + P],
                                     rhs=wo_sb[:, jc, nh * SBLK:(nh + 1) * SBLK],
                                     start=(jc == 0), stop=(jc == 1))
            nc.vector.tensor_copy(ob_holder[0][:, st, :],
                                  po[:].rearrange("p u q -> p (u q)"))
            if b == NBLK - 1:
                nc.gpsimd.dma_start(out_d[b][:, st * D:(st + 1) * D],
                                    ob_holder[0][:, st, :])
            elif st == 3:
                nc.gpsimd.dma_start(out_d[b],
                                    ob_holder[0][:].rearrange("p a b -> p (a b)"))

        # ---- emission schedule ----
        # Minimal prologue: K/Q chunk 0 land first and block 0 pair 0 starts
        # immediately; V chunk 0 feeds the (2-step-trailing) ctx matmuls.
        # Everything else — K/V chunks 1-3, Q chunks 1-3, out-proj,
        # normalize — drains into the per-step slack of the blocks as small
        # (<=4-matmul) filler units, slotted so each unit is emitted strictly
        # before its consumer but late enough that its DMA has landed (a
        # piece waiting on DMA at the PE queue head stalls everything).
        kt0 = load_chunk("kt", 0)
        nc.gpsimd.dma_start(wq_sb[:].rearrange("p a b -> p (a b)"), d["wq"])
        qt0 = load_chunk("qt", 0, nc.gpsimd)
        vt0 = load_chunk("vt", 0, nc.gpsimd)
        proj_qk(kt0, wk_sb, kwt, BK_OFF, 0, 0)
        proj_qk(kt0, wk_sb, kwt, BK_OFF, 0, 1)
        proj_qk(qt0, wq_sb, qwt, BQ_OFF, 0, 0)
        proj_qk(qt0, wq_sb, qwt, BQ_OFF, 0, 1)
        kts = {1: load_chunk("kt", 1)}
        vts = {0: vt0, 1: load_chunk("vt", 1, nc.gpsimd)}
        kts[2] = load_chunk("kt", 2)
        vts[2] = load_chunk("vt", 2, nc.gpsimd)
        nc.sync.dma_start(wo_sb[:].rearrange("p a b -> p (a b)"), d["wo"])
        qts = {0: qt0}

        def k_fillers(ci, pair, s0):
            """3 pieces: dc 0-2, 3-5, 6-7+bias+evict."""
            holder = [None, None]
            return [(s0 + j, lambda r=tuple(rr), h=holder, c=ci, p=pair:
                     proj_qk_piece(kts[c], wk_sb, kwt, BK_OFF, c, p, r, h))
                    for j, rr in enumerate(([0, 1, 2], [3, 4, 5], [6, 7]))]

        def q_fillers(ci, s0):
            out = []
            slot = s0
            for pair in range(2):
                holder = [None, None]
                for rr in ([0, 1, 2], [3, 4, 5], [6, 7]):
                    out.append((slot, lambda p=pair, r=tuple(rr), h=holder, c=ci:
                                proj_qk_piece(qts[c], wq_sb, qwt, BQ_OFF, c, p, r, h)))
                    slot += 1
            return out

        def v_fillers(ci, s0):
            """2 pieces per t-tile at slots (s0+c, s0+c+1): piece 2 lands one
            step before ctx(tt) consumes the tile (ctx trails by 2)."""
            out = []
            for c in range(4):
                holder = [None]
                for j, rr in enumerate(([0, 1, 2, 3], [4, 5, 6, 7])):
                    out.append((s0 + c + j,
                                lambda cc=c, r=tuple(rr), h=holder, ci_=ci:
                                proj_v_piece(vts[ci_], ci_, cc, r, h)))
            return out

        # b0p0: stream in K chunks 1-3 (pair 0) and all V chunks in slack.
        fill = sorted(
            v_fillers(0, 0)
            + k_fillers(1, 0, 1)
            + k_fillers(1, 1, 2)  # pair-1 c1 must fully consume kt1 before
                                  # vt3's DMA (slot 5) recycles its buffer
            + [(5, lambda: kts.__setitem__(3, load_chunk("kt", 3))),
               (6, lambda: vts.__setitem__(3, load_chunk("vt", 3, nc.gpsimd)))]
            + k_fillers(2, 0, 4)
            + v_fillers(1, 4)
            + k_fillers(3, 0, 8)
            + v_fillers(2, 8)
            + v_fillers(3, 12),
            key=lambda x: x[0])
        ct = attn_block(0, 0, fill)
        prev = (0, 0, ct)

        # remaining 7 pair-blocks
        for b in range(NBLK):
            for pair in range(2):
                if b == 0 and pair == 0:
                    continue
                fill = []
                pp, pb, pct = prev
                fill.append((0, lambda p=pp, bb=pb, c=pct: attn_normalize(p, bb, c, 0)))
                fill.append((2, lambda p=pp, bb=pb, c=pct: attn_normalize(p, bb, c, 1)))
                if b == 0 and pair == 1:
                    # pair-1 K projections (chunk ci first needed by scores
                    # at step 4*ci, so pieces must sit at slots <= 4*ci-1)
                    fill += k_fillers(2, 1, 4)
                    fill += k_fillers(3, 1, 7)
                if pair == 1 and b < NBLK - 1:
                    # next block's Q chunk: DMA now, project late in block
                    qts[b + 1] = load_chunk("qt", b + 1)
                    fill += q_fillers(b + 1, 10)
                if pair == 0 and b > 0:
                    for st in range(4):
                        fill.append((5 + st * 3, lambda bb=b - 1, s=st:
                                     out_proj_st(bb, s)))
                fill.sort(key=lambda x: x[0])
                ct = attn_block(pair, b, fill)
                prev = (pair, b, ct)
        attn_normalize(1, NBLK - 1, ct, 0)
        attn_normalize(1, NBLK - 1, ct, 1)
        for st in range(4):
            out_proj_st(NBLK - 1, st)

    nc.compile()
    return nc


_NC_CACHE = None


def _get_nc():
    global _NC_CACHE
    if _NC_CACHE is None:
        _NC_CACHE = _build_nc()
    return _NC_CACHE


def _chunked(xT):
    """[D, S] -> [NBLK, P, NDC*SBLK]: chunk tile (p, dc, s) = xT[dc*128+p,
    ci*512+s], laid out so each partition's chunk row is contiguous."""
    x = xT.reshape(NDC, P, NBLK, SBLK).transpose(2, 1, 0, 3)
    return np.ascontiguousarray(x.reshape(NBLK, P, NDC * SBLK)).astype(np.float16)


def _wpack(w, cols):
    """[D, cols] -> [P, NDC*cols] with (p, dc, m) = w[dc*128+p, m]."""
    x = w.reshape(NDC, P, cols).transpose(1, 0, 2)
    return np.ascontiguousarray(x.reshape(P, NDC * cols)).astype(np.float16)


def kernel(Q, K, V, Wq, bq, Wk, bk, Wv, bv, Wo, bo, _trace=False, _trace_kwargs=None):
    nc = _get_nc()
    qt_h = [_chunked(np.asarray(Q[b]).T) for b in range(B)]
    kt_h = [_chunked(np.asarray(K[b]).T) for b in range(B)]
    vt_h = [_chunked(np.asarray(V[b]).T) for b in range(B)]
    onesdv = np.ones((1, DV), dtype=np.float32)

    in_maps = []
    for c in range(N_CORES):
        b, g = c % B, c // B
        hs = list(range(g * HPC, (g + 1) * HPC))
        wq_p = np.concatenate([Wq[h] for h in hs], axis=1)
        wk_p = np.concatenate([Wk[h] for h in hs], axis=1)
        wv_e = np.zeros((D, NV), dtype=np.float32)
        bv_e = np.zeros(NV, dtype=np.float32)
        for i, h in enumerate(hs):
            wv_e[:, i * (DV + 1):i * (DV + 1) + DV] = Wv[h] * SCALE
            bv_e[i * (DV + 1):i * (DV + 1) + DV] = bv[h] * SCALE
            bv_e[i * (DV + 1) + DV] = 1.0
        brow = np.zeros((1, BROW_W), dtype=np.float32)
        brow[0, ONES_OFF:ONES_OFF + SBLK] = 1.0
        brow[0, BQ_OFF:BQ_OFF + 2 * P] = np.concatenate([bq[h] for h in hs])
        brow[0, BK_OFF:BK_OFF + 2 * P] = np.concatenate([bk[h] for h in hs])
        brow[0, BVE_OFF:BVE_OFF + NV] = bv_e
        wo_g = np.asarray(Wo[g * HPC * DV:(g + 1) * HPC * DV])  # [256, 1024]
        wo_p = wo_g.reshape(2, P, D).transpose(1, 0, 2).reshape(P, 2 * D)
        in_maps.append({
            "qt": qt_h[b], "kt": kt_h[b], "vt": vt_h[b],
            "wq": _wpack(wq_p, 2 * P),
            "wk": _wpack(wk_p, 2 * P),
            "wv": _wpack(wv_e, NV),
            "wo": np.ascontiguousarray(wo_p).astype(np.float16),
            "brow": brow.astype(np.float16),
            "onesdv": onesdv,
        })

    kw = {}
    if _trace:
        kw = dict(trace=True, **(_trace_kwargs or {}))
    res = run_bass_kernel_spmd(nc, in_maps, core_ids=list(range(N_CORES)), **kw)

    out = np.zeros((B, S, D), dtype=np.float32)
    for c in range(N_CORES):
        o = np.asarray(res.results[c]["out"], dtype=np.float32)
        # [NBLK, P, 4*D]: row s = b*512 + st*128 + p
        o = o.reshape(NBLK, P, 4, D).transpose(0, 2, 1, 3).reshape(S, D)
        out[c % B] += o
    out += bo[None, None, :]
    if _trace:
        return out, res
    return out


# revision 55
# speedup vs baseline: 1.0430x; 1.0430x over previous
"""Multi-head attention (B=2, S=2048, D=1024, H=16, dk=dv=64) on 8 TRN2 cores.

Sharding: core c -> batch b = c % 2, head-group g = c // 2 (heads 4g..4g+3).
Each core computes its 4 heads' attention for one batch plus the partial
output projection; the host sums the 4 partials per batch and adds bo.

v2 design (vs the transpose-heavy v1): the whole input path is fp16
(halves HBM traffic; fp16's 11-bit mantissa keeps scores to ~1e-3), the
V projection is computed directly in natural [t, dv] layout (lhsT = the
V chunk itself), and every projection bias is a rank-1 K=1 matmul into
the accumulating PSUM so all PSUM->SBUF evictions are pure DVE copies.
The ACT engine then does nothing but the softmax exp, which is its hard
floor: (1024+352)/1.2GHz per [128,1024] tile, ~147us over the 128 tiles.
The attention pipeline runs one t-tile per step (scores -> exp -> ctx
trailing 2 steps) with a 3-deep scores-PSUM rotation so the PE can run
ahead of ACT and never bubbles long enough to re-throttle the HAM clock
gate. K/V/Q chunks stream in while block 0 is already computing; Q-proj,
out-proj and normalize work drains into later blocks' per-step slack.
"""
import os
import sys

sys.path.insert(0, "/opt/trn_rl_repo")
os.environ.setdefault("JAX_PLATFORMS", "axon,cpu")

from contextlib import ExitStack

import numpy as np

import concourse.bacc as bacc
import concourse.tile as tile
from concourse import mybir
from concourse.bass_utils import run_bass_kernel_spmd

FP16 = mybir.dt.float16
BF16 = mybir.dt.bfloat16
FP32 = mybir.dt.float32
FP32R = mybir.dt.float32r

B, S, D = 2, 2048, 1024
H, DK, DV = 16, 64, 64
N_CORES = 8
HPC = H // (N_CORES // B)  # heads per core = 4
P = 128
SBLK = 512                # s-block (free dim of scores matmuls)
NBLK = S // SBLK          # 4
NTT = S // P              # 16 t-tiles
NDC = D // P              # 8 contraction chunks
NV = HPC * (DV + 1)       # 260 (64 V cols + 1 denominator-ones col per head)
SCALE = 1.0 / (DK * 2.0)  # folded into Wv/bv
# brow packing offsets (one [1, 1284] fp16 row of constants)
ONES_OFF, BQ_OFF, BK_OFF, BVE_OFF = 0, 512, 768, 1024
BROW_W = 1284


def _build_nc():
    nc = bacc.Bacc("TRN2", target_bir_lowering=False, debug=False,
                   num_devices=N_CORES)
    # All bulk tensors are host-permuted so every DMA line is one long
    # contiguous row per partition (128 descriptors per transfer, not 1024
    # 1KB ones — descriptor generation was serializing the DMA queues).
    d = {}
    for name, shape, dt in [
        ("qt", [NBLK, P, NDC * SBLK], FP16), ("kt", [NBLK, P, NDC * SBLK], FP16),
        ("vt", [NBLK, P, NDC * SBLK], FP16),
        ("wq", [P, NDC * 2 * P], FP16), ("wk", [P, NDC * 2 * P], FP16),
        ("wv", [P, NDC * NV], FP16), ("wo", [P, 2 * D], FP16),
        ("brow", [1, BROW_W], FP16), ("onesdv", [1, DV], FP32),
    ]:
        d[name] = nc.dram_tensor(name, shape, dt, kind="ExternalInput").ap()
    out_d = nc.dram_tensor("out", [NBLK, P, 4 * D], FP16, kind="ExternalOutput").ap()

    with tile.TileContext(nc) as tc, ExitStack() as ctx:
        const = ctx.enter_context(tc.tile_pool(name="const", bufs=1))
        wpool = ctx.enter_context(tc.tile_pool(name="wpool", bufs=1))
        xtp = ctx.enter_context(tc.tile_pool(name="xtp", bufs=1))
        projp = ctx.enter_context(tc.tile_pool(name="projp", bufs=1))
        expp = ctx.enter_context(tc.tile_pool(name="expp", bufs=1))
        ctxp = ctx.enter_context(tc.tile_pool(name="ctxp", bufs=1))
        outp = ctx.enter_context(tc.tile_pool(name="outp", bufs=2))
        smallp = ctx.enter_context(tc.tile_pool(name="smallp", bufs=2))
        psum = ctx.enter_context(tc.tile_pool(name="psum", bufs=1, space="PSUM"))

        # ---- ACT table warm-up: a 2-elem exp triggers ACT_TABLE_LOAD
        # while the first DMAs are still in flight.
        dummy = smallp.tile([1, 2], FP32, tag="dmy")
        dummy2 = smallp.tile([1, 2], FP32, tag="dmy2")
        nc.vector.memset(dummy[:], 0.0)
        nc.scalar.activation(dummy2[:], dummy[:],
                             mybir.ActivationFunctionType.Exp)

        # ---- constants / weights (sync queue: K/Q path; gpsimd queue: V) ----
        brow = const.tile([1, BROW_W], FP16)
        nc.sync.dma_start(brow[:], d["brow"])
        wk_sb = wpool.tile([P, NDC, 2 * P], FP16)
        nc.sync.dma_start(wk_sb[:].rearrange("p a b -> p (a b)"), d["wk"])
        onesdv = const.tile([1, DV], FP32R)
        nc.gpsimd.dma_start(onesdv[:], d["onesdv"].bitcast(FP32R))
        wv_sb = wpool.tile([P, NDC, NV], FP16)
        nc.gpsimd.dma_start(wv_sb[:].rearrange("p a b -> p (a b)"), d["wv"])
        wq_sb = wpool.tile([P, NDC, 2 * P], FP16)
        wo_sb = wpool.tile([P, 2, D], FP16)

        # ---- persistent activation tiles ----
        kwt = [projp.tile([P, S], FP16, tag=f"kwt{p_}", name=f"kwt{p_}") for p_ in range(2)]
        qwt = [projp.tile([P, S], FP16, tag=f"qwt{p_}", name=f"qwt{p_}") for p_ in range(2)]
        vw = projp.tile([P, NTT, NV], BF16, tag="vw")
        ctx_t = [ctxp.tile([P, S], FP16, tag=f"ctx{p_}", name=f"ctx{p_}") for p_ in range(2)]

        def load_chunk(name, ci, eng=None):
            # kt/qt ride the sync DMA queue, vt the gpsimd queue: two
            # parallel streams halve the DMA-gated prologue.
            xt = xtp.tile([P, NDC, SBLK], FP16, tag="xt", name="xt", bufs=5)
            (eng or nc.sync).dma_start(
                xt[:].rearrange("p a b -> p (a b)"), d[name][ci])
            return xt

        def proj_qk_piece(xt, w_sb, dst, bias_off, ci, pair, dc_range, pq_holder):
            """Part of one head-pair x one 512-s-chunk projection; the final
            piece adds the rank-1 bias and DVE-evicts to fp16 SBUF."""
            if dc_range[0] == 0:
                pq_holder[pair] = psum.tile([P, 2, SBLK], FP32, tag="sc",
                                            name="pq", bufs=3)
            pq = pq_holder[pair]
            for dc in dc_range:
                nc.tensor.matmul(pq[:, 0, :], lhsT=w_sb[:, dc, pair * P:(pair + 1) * P],
                                 rhs=xt[:, dc, :], start=(dc == 0), stop=False)
            if dc_range[-1] == NDC - 1:
                nc.tensor.matmul(
                    pq[:, 0, :],
                    lhsT=brow[:, bias_off + pair * P:bias_off + (pair + 1) * P],
                    rhs=brow[:, ONES_OFF:ONES_OFF + SBLK],
                    start=False, stop=True)
                nc.vector.tensor_copy(dst[pair][:, ci * SBLK:(ci + 1) * SBLK],
                                      pq[:, 0, :])

        def proj_qk(xt, w_sb, dst, bias_off, ci, pair):
            h = [None, None]
            proj_qk_piece(xt, w_sb, dst, bias_off, ci, pair, range(NDC), h)

        def proj_v_piece(xt, ci, c, dc_range, pv_holder):
            """Part of one t-tile of the natural-layout V projection."""
            tt = ci * (SBLK // P) + c
            if dc_range[0] == 0:
                pv_holder[0] = psum.tile([P, 2, SBLK], FP32, tag="sc",
                                         name="pv", bufs=3)
            pv = pv_holder[0]
            for dc in dc_range:
                nc.tensor.matmul(pv[:, 0, 0:NV], lhsT=xt[:, dc, c * P:(c + 1) * P],
                                 rhs=wv_sb[:, dc, :], start=(dc == 0), stop=False)
            if dc_range[-1] == NDC - 1:
                nc.tensor.matmul(pv[:, 0, 0:NV], lhsT=brow[:, ONES_OFF:ONES_OFF + P],
                                 rhs=brow[:, BVE_OFF:BVE_OFF + NV],
                                 start=False, stop=True)
                nc.vector.tensor_copy(vw[:, tt, :], pv[:, 0, 0:NV])

        def proj_v_tt(xt, ci, c):
            h = [None]
            proj_v_piece(xt, ci, c, range(NDC), h)

        def attn_block(pair, b, fillers):
            """Per-t-tile pipeline: scores(k) -> exp(k) -> ctx(k-2).
            One 2-bank scores PSUM per step (hp0 | hp1), 3-deep rotation;
            exp is a single FD=1024 ACT instruction. `fillers` is a list of
            (slot, fn); fn is emitted when the step index reaches slot."""
            ct = [psum.tile([DV + 1, SBLK], FP32, tag=f"ct{hp}", name=f"ct{hp}")
                  for hp in range(2)]
            exs = {}
            for k in range(NTT + 2):
                if k < NTT:
                    sc = psum.tile([P, 2, SBLK], FP32, tag="sc", name="sc", bufs=3)
                    for hp in range(2):
                        lo, hi = hp * DK, (hp + 1) * DK
                        nc.tensor.matmul(
                            sc[:, hp, :],
                            lhsT=kwt[pair][lo:hi, k * P:(k + 1) * P],
                            rhs=qwt[pair][lo:hi, b * SBLK:(b + 1) * SBLK],
                            start=True, stop=True)
                    ex = expp.tile([P, 2, SBLK], BF16, tag="ex", name="ex", bufs=4)
                    nc.scalar.activation(ex[:], sc[:],
                                         mybir.ActivationFunctionType.Exp)
                    exs[k] = ex
                while fillers and fillers[0][0] <= k:
                    fillers.pop(0)[1]()
                kc = k - 2
                if kc >= 0:
                    ex = exs.pop(kc)
                    for hp in range(2):
                        hh = 2 * pair + hp
                        nc.tensor.matmul(
                            ct[hp][:], lhsT=vw[:, kc, hh * (DV + 1):(hh + 1) * (DV + 1)],
                            rhs=ex[:, hp, :],
                            start=(kc == 0), stop=(kc == NTT - 1))
            return ct

        def attn_normalize(pair, b, ct, hp):
            # ctx = ct[0:64] * (1 / ct[64]) row-broadcast; fp16 out
            den = smallp.tile([1, SBLK], FP32R, tag="den")
            nc.vector.tensor_copy(den[:], ct[hp][DV:DV + 1, :])
            rb = psum.tile([P, 2, SBLK], FP32, tag="sc", name="rb", bufs=3)
            nc.tensor.matmul(rb[0:DV, 0, :], lhsT=onesdv[:],
                             rhs=den[:], start=True, stop=True)
            rcp = smallp.tile([DV, SBLK], FP32, tag="rcp")
            nc.vector.reciprocal_approx_fast(rcp[:], rb[0:DV, 0, :])
            nc.vector.tensor_mul(
                ctx_t[pair][hp * DV:(hp + 1) * DV, b * SBLK:(b + 1) * SBLK],
                ct[hp][0:DV, :], rcp[:])

        ob_holder = [None]

        def out_proj_st(b, st):
            """One s-tile of the output projection: [128 s, 1024 D] via 4
            N=512 matmuls (fp16 moving operand caps at 512); the 4 s-tiles
            of a block stage into one SBUF tile DMA'd out as a single 1MB
            transfer on the gpsimd queue."""
            off = b * SBLK + st * P
            if st == 0:
                ob_holder[0] = outp.tile([P, 4, D], FP16, tag="ob", name="ob")
            po = psum.tile([P, 2, SBLK], FP32, tag="sc", name="po", bufs=3)
            for nh in range(2):
                for jc in range(2):
                    nc.tensor.matmul(po[:, nh, :],
                                     lhsT=ctx_t[jc][:, off:off + P],
                                     rhs=wo_sb[:, jc, nh * SBLK:(nh + 1) * SBLK],
                                     start=(jc == 0), stop=(jc == 1))
            nc.vector.tensor_copy(ob_holder[0][:, st, :],
                                  po[:].rearrange("p u q -> p (u q)"))
            if b == NBLK - 1:
                nc.gpsimd.dma_start(out_d[b][:, st * D:(st + 1) * D],
                                    ob_holder[0][:, st, :])
            elif st == 3:
                nc.gpsimd.dma_start(out_d[b],
                                    ob_holder[0][:].rearrange("p a b -> p (a b)"))

        # ---- emission schedule ----
        # Minimal prologue: K/Q chunk 0 land first and block 0 pair 0 starts
        # immediately; V chunk 0 feeds the (2-step-trailing) ctx matmuls.
        # Everything else — K/V chunks 1-3, Q chunks 1-3, out-proj,
        # normalize — drains into the per-step slack of the blocks as small
        # (<=4-matmul) filler units, slotted so each unit is emitted strictly
        # before its consumer but late enough that its DMA has landed (a
        # piece waiting on DMA at the PE queue head stalls everything).
        kt0 = load_chunk("kt", 0)
        nc.gpsimd.dma_start(wq_sb[:].rearrange("p a b -> p (a b)"), d["wq"])
        qt0 = load_chunk("qt", 0, nc.gpsimd)
        vt0 = load_chunk("vt", 0, nc.gpsimd)
        proj_qk(kt0, wk_sb, kwt, BK_OFF, 0, 0)
        proj_qk(kt0, wk_sb, kwt, BK_OFF, 0, 1)
        proj_qk(qt0, wq_sb, qwt, BQ_OFF, 0, 0)
        proj_qk(qt0, wq_sb, qwt, BQ_OFF, 0, 1)
        kts = {1: load_chunk("kt", 1)}
        vts = {0: vt0, 1: load_chunk("vt", 1, nc.gpsimd)}
        kts[2] = load_chunk("kt", 2)
        vts[2] = load_chunk("vt", 2, nc.gpsimd)
        nc.sync.dma_start(wo_sb[:].rearrange("p a b -> p (a b)"), d["wo"])
        qts = {0: qt0}

        def k_fillers(ci, pair, s0):
            """3 pieces: dc 0-2, 3-5, 6-7+bias+evict."""
            holder = [None, None]
            return [(s0 + j, lambda r=tuple(rr), h=holder, c=ci, p=pair:
                     proj_qk_piece(kts[c], wk_sb, kwt, BK_OFF, c, p, r, h))
                    for j, rr in enumerate(([0, 1, 2], [3, 4, 5], [6, 7]))]

        def q_fillers(ci, s0):
            out = []
            slot = s0
            for pair in range(2):
                holder = [None, None]
                for rr in ([0, 1, 2], [3, 4, 5], [6, 7]):
                    out.append((slot, lambda p=pair, r=tuple(rr), h=holder, c=ci:
                                proj_qk_piece(qts[c], wq_sb, qwt, BQ_OFF, c, p, r, h)))
                    slot += 1
            return out

        def v_fillers(ci, s0):
            """2 pieces per t-tile at slots (s0+c, s0+c+1): piece 2 lands one
            step before ctx(tt) consumes the tile (ctx trails by 2)."""
            out = []
            for c in range(4):
                holder = [None]
                for j, rr in enumerate(([0, 1, 2, 3], [4, 5, 6, 7])):
                    out.append((s0 + c + j,
                                lambda cc=c, r=tuple(rr), h=holder, ci_=ci:
                                proj_v_piece(vts[ci_], ci_, cc, r, h)))
            return out

        # b0p0: stream in K chunks 1-3 (pair 0) and all V chunks in slack.
        fill = sorted(
            v_fillers(0, 0)
            + k_fillers(1, 0, 1)
            + k_fillers(1, 1, 2)  # pair-1 c1 must fully consume kt1 before
                                  # vt3's DMA (slot 5) recycles its buffer
            + [(5, lambda: kts.__setitem__(3, load_chunk("kt", 3))),
               (6, lambda: vts.__setitem__(3, load_chunk("vt", 3, nc.gpsimd)))]
            + k_fillers(2, 0, 4)
            + v_fillers(1, 4)
            + k_fillers(3, 0, 8)
            + v_fillers(2, 8)
            + v_fillers(3, 12),
            key=lambda x: x[0])
        ct = attn_block(0, 0, fill)
        prev = (0, 0, ct)

        # remaining 7 pair-blocks
        for b in range(NBLK):
            for pair in range(2):
                if b == 0 and pair == 0:
                    continue
                fill = []
                pp, pb, pct = prev
                fill.append((0, lambda p=pp, bb=pb, c=pct: attn_normalize(p, bb, c, 0)))
                fill.append((1, lambda p=pp, bb=pb, c=pct: attn_normalize(p, bb, c, 1)))
                if b == 0 and pair == 1:
                    # pair-1 K projections (chunk ci first needed by scores
                    # at step 4*ci, so pieces must sit at slots <= 4*ci-1)
                    fill += k_fillers(2, 1, 4)
                    fill += k_fillers(3, 1, 7)
                if pair == 1 and b < NBLK - 1:
                    # next block's Q chunk: DMA now, project late in block
                    qts[b + 1] = load_chunk("qt", b + 1)
                    fill += q_fillers(b + 1, 10)
                if pair == 0 and b > 0:
                    for st in range(4):
                        fill.append((4 + st * 3, lambda bb=b - 1, s=st:
                                     out_proj_st(bb, s)))
                fill.sort(key=lambda x: x[0])
                ct = attn_block(pair, b, fill)
                prev = (pair, b, ct)
        attn_normalize(1, NBLK - 1, ct, 0)
        attn_normalize(1, NBLK - 1, ct, 1)
        for st in range(4):
            out_proj_st(NBLK - 1, st)

    nc.compile()
    return nc


_NC_CACHE = None


def _get_nc():
    global _NC_CACHE
    if _NC_CACHE is None:
        _NC_CACHE = _build_nc()
    return _NC_CACHE


def _chunked(xT):
    """[D, S] -> [NBLK, P, NDC*SBLK]: chunk tile (p, dc, s) = xT[dc*128+p,
    ci*512+s], laid out so each partition's chunk row is contiguous."""
    x = xT.reshape(NDC, P, NBLK, SBLK).transpose(2, 1, 0, 3)
    return np.ascontiguousarray(x.reshape(NBLK, P, NDC * SBLK)).astype(np.float16)


def _wpack(w, cols):
    """[D, cols] -> [P, NDC*cols] with (p, dc, m) = w[dc*128+p, m]."""
    x = w.reshape(NDC, P, cols).transpose(1, 0, 2)
    return np.ascontiguousarray(x.reshape(P, NDC * cols)).astype(np.float16)


def kernel(Q, K, V, Wq, bq, Wk, bk, Wv, bv, Wo, bo, _trace=False, _trace_kwargs=None):
    nc = _get_nc()
    qt_h = [_chunked(np.asarray(Q[b]).T) for b in range(B)]
    kt_h = [_chunked(np.asarray(K[b]).T) for b in range(B)]
    vt_h = [_chunked(np.asarray(V[b]).T) for b in range(B)]
    onesdv = np.ones((1, DV), dtype=np.float32)

    in_maps = []
    for c in range(N_CORES):
        b, g = c % B, c // B
        hs = list(range(g * HPC, (g + 1) * HPC))
        wq_p = np.concatenate([Wq[h] for h in hs], axis=1)
        wk_p = np.concatenate([Wk[h] for h in hs], axis=1)
        wv_e = np.zeros((D, NV), dtype=np.float32)
        bv_e = np.zeros(NV, dtype=np.float32)
        for i, h in enumerate(hs):
            wv_e[:, i * (DV + 1):i * (DV + 1) + DV] = Wv[h] * SCALE
            bv_e[i * (DV + 1):i * (DV + 1) + DV] = bv[h] * SCALE
            bv_e[i * (DV + 1) + DV] = 1.0
        brow = np.zeros((1, BROW_W), dtype=np.float32)
        brow[0, ONES_OFF:ONES_OFF + SBLK] = 1.0
        brow[0, BQ_OFF:BQ_OFF + 2 * P] = np.concatenate([bq[h] for h in hs])
        brow[0, BK_OFF:BK_OFF + 2 * P] = np.concatenate([bk[h] for h in hs])
        brow[0, BVE_OFF:BVE_OFF + NV] = bv_e
        wo_g = np.asarray(Wo[g * HPC * DV:(g + 1) * HPC * DV])  # [256, 1024]
        wo_p = wo_g.reshape(2, P, D).transpose(1, 0, 2).reshape(P, 2 * D)
        in_maps.append({
            "qt": qt_h[b], "kt": kt_h[b], "vt": vt_h[b],
            "wq": _wpack(wq_p, 2 * P),
            "wk": _wpack(wk_p, 2 * P),
            "wv": _wpack(wv_e, NV),
            "wo": np.ascontiguousarray(wo_p).astype(np.float16),
            "brow": brow.astype(np.float16),
            "onesdv": onesdv,
        })

    kw = {}
    if _trace:
        kw = dict(trace=True, **(_trace_kwargs or {}))
    res = run_bass_kernel_spmd(nc, in_maps, core_ids=list(range(N_CORES)), **kw)

    out = np.zeros((B, S, D), dtype=np.float32)
    for c in range(N_CORES):
        o = np.asarray(res.results[c]["out"], dtype=np.float32)
        # [NBLK, P, 4*D]: row s = b*512 + st*128 + p
        o = o.reshape(NBLK, P, 4, D).transpose(0, 2, 1, 3).reshape(S, D)
        out[c % B] += o
    out += bo[None, None, :]
    if _trace:
        return out, res
    return out


# revision 56
# speedup vs baseline: 1.0472x; 1.0040x over previous
"""Multi-head attention (B=2, S=2048, D=1024, H=16, dk=dv=64) on 8 TRN2 cores.

Sharding: core c -> batch b = c % 2, head-group g = c // 2 (heads 4g..4g+3).
Each core computes its 4 heads' attention for one batch plus the partial
output projection; the host sums the 4 partials per batch and adds bo.

v2 design (vs the transpose-heavy v1): the whole input path is fp16
(halves HBM traffic; fp16's 11-bit mantissa keeps scores to ~1e-3), the
V projection is computed directly in natural [t, dv] layout (lhsT = the
V chunk itself), and every projection bias is a rank-1 K=1 matmul into
the accumulating PSUM so all PSUM->SBUF evictions are pure DVE copies.
The ACT engine then does nothing but the softmax exp, which is its hard
floor: (1024+352)/1.2GHz per [128,1024] tile, ~147us over the 128 tiles.
The attention pipeline runs one t-tile per step (scores -> exp -> ctx
trailing 2 steps) with a 3-deep scores-PSUM rotation so the PE can run
ahead of ACT and never bubbles long enough to re-throttle the HAM clock
gate. K/V/Q chunks stream in while block 0 is already computing; Q-proj,
out-proj and normalize work drains into later blocks' per-step slack.
"""
import os
import sys

sys.path.insert(0, "/opt/trn_rl_repo")
os.environ.setdefault("JAX_PLATFORMS", "axon,cpu")

from contextlib import ExitStack

import numpy as np

import concourse.bacc as bacc
import concourse.tile as tile
from concourse import mybir
from concourse.bass_utils import run_bass_kernel_spmd

FP16 = mybir.dt.float16
BF16 = mybir.dt.bfloat16
FP32 = mybir.dt.float32
FP32R = mybir.dt.float32r

B, S, D = 2, 2048, 1024
H, DK, DV = 16, 64, 64
N_CORES = 8
HPC = H // (N_CORES // B)  # heads per core = 4
P = 128
SBLK = 512                # s-block (free dim of scores matmuls)
NBLK = S // SBLK          # 4
NTT = S // P              # 16 t-tiles
NDC = D // P              # 8 contraction chunks
NV = HPC * (DV + 1)       # 260 (64 V cols + 1 denominator-ones col per head)
SCALE = 1.0 / (DK * 2.0)  # folded into Wv/bv
# brow packing offsets (one [1, 1284] fp16 row of constants)
ONES_OFF, BQ_OFF, BK_OFF, BVE_OFF = 0, 512, 768, 1024
BROW_W = 1284


def _build_nc():
    nc = bacc.Bacc("TRN2", target_bir_lowering=False, debug=False,
                   num_devices=N_CORES)
    # All bulk tensors are host-permuted so every DMA line is one long
    # contiguous row per partition (128 descriptors per transfer, not 1024
    # 1KB ones — descriptor generation was serializing the DMA queues).
    d = {}
    for name, shape, dt in [
        ("qt", [NBLK, P, NDC * SBLK], FP16), ("kt", [NBLK, P, NDC * SBLK], FP16),
        ("vt", [NBLK, P, NDC * SBLK], FP16),
        ("wq", [P, NDC * 2 * P], FP16), ("wk", [P, NDC * 2 * P], FP16),
        ("wv", [P, NDC * NV], FP16), ("wo", [P, 2 * D], FP16),
        ("brow", [1, BROW_W], FP16), ("onesdv", [1, DV], FP32),
    ]:
        d[name] = nc.dram_tensor(name, shape, dt, kind="ExternalInput").ap()
    out_d = nc.dram_tensor("out", [NBLK, P, 4 * D], FP16, kind="ExternalOutput").ap()

    with tile.TileContext(nc) as tc, ExitStack() as ctx:
        const = ctx.enter_context(tc.tile_pool(name="const", bufs=1))
        wpool = ctx.enter_context(tc.tile_pool(name="wpool", bufs=1))
        xtp = ctx.enter_context(tc.tile_pool(name="xtp", bufs=1))
        projp = ctx.enter_context(tc.tile_pool(name="projp", bufs=1))
        expp = ctx.enter_context(tc.tile_pool(name="expp", bufs=1))
        ctxp = ctx.enter_context(tc.tile_pool(name="ctxp", bufs=1))
        outp = ctx.enter_context(tc.tile_pool(name="outp", bufs=2))
        smallp = ctx.enter_context(tc.tile_pool(name="smallp", bufs=2))
        psum = ctx.enter_context(tc.tile_pool(name="psum", bufs=1, space="PSUM"))

        # ---- ACT table warm-up: a 2-elem exp triggers ACT_TABLE_LOAD
        # while the first DMAs are still in flight.
        dummy = smallp.tile([1, 2], FP32, tag="dmy")
        dummy2 = smallp.tile([1, 2], FP32, tag="dmy2")
        nc.vector.memset(dummy[:], 0.0)
        nc.scalar.activation(dummy2[:], dummy[:],
                             mybir.ActivationFunctionType.Exp)

        # ---- constants / weights (sync queue: K/Q path; gpsimd queue: V) ----
        brow = const.tile([1, BROW_W], FP16)
        nc.sync.dma_start(brow[:], d["brow"])
        wk_sb = wpool.tile([P, NDC, 2 * P], FP16)
        nc.sync.dma_start(wk_sb[:].rearrange("p a b -> p (a b)"), d["wk"])
        onesdv = const.tile([1, DV], FP32R)
        nc.gpsimd.dma_start(onesdv[:], d["onesdv"].bitcast(FP32R))
        wv_sb = wpool.tile([P, NDC, NV], FP16)
        nc.gpsimd.dma_start(wv_sb[:].rearrange("p a b -> p (a b)"), d["wv"])
        wq_sb = wpool.tile([P, NDC, 2 * P], FP16)
        wo_sb = wpool.tile([P, 2, D], FP16)

        # ---- persistent activation tiles ----
        kwt = [projp.tile([P, S], FP16, tag=f"kwt{p_}", name=f"kwt{p_}") for p_ in range(2)]
        qwt = [projp.tile([P, S], FP16, tag=f"qwt{p_}", name=f"qwt{p_}") for p_ in range(2)]
        vw = projp.tile([P, NTT, NV], BF16, tag="vw")
        ctx_t = [ctxp.tile([P, S], FP16, tag=f"ctx{p_}", name=f"ctx{p_}") for p_ in range(2)]

        def load_chunk(name, ci, eng=None):
            # kt/qt ride the sync DMA queue, vt the gpsimd queue: two
            # parallel streams halve the DMA-gated prologue.
            xt = xtp.tile([P, NDC, SBLK], FP16, tag="xt", name="xt", bufs=5)
            (eng or nc.sync).dma_start(
                xt[:].rearrange("p a b -> p (a b)"), d[name][ci])
            return xt

        def proj_qk_piece(xt, w_sb, dst, bias_off, ci, pair, dc_range, pq_holder):
            """Part of one head-pair x one 512-s-chunk projection; the final
            piece adds the rank-1 bias and DVE-evicts to fp16 SBUF."""
            if dc_range[0] == 0:
                pq_holder[pair] = psum.tile([P, 2, SBLK], FP32, tag="sc",
                                            name="pq", bufs=3)
            pq = pq_holder[pair]
            for dc in dc_range:
                nc.tensor.matmul(pq[:, 0, :], lhsT=w_sb[:, dc, pair * P:(pair + 1) * P],
                                 rhs=xt[:, dc, :], start=(dc == 0), stop=False)
            if dc_range[-1] == NDC - 1:
                nc.tensor.matmul(
                    pq[:, 0, :],
                    lhsT=brow[:, bias_off + pair * P:bias_off + (pair + 1) * P],
                    rhs=brow[:, ONES_OFF:ONES_OFF + SBLK],
                    start=False, stop=True)
                nc.vector.tensor_copy(dst[pair][:, ci * SBLK:(ci + 1) * SBLK],
                                      pq[:, 0, :])

        def proj_qk(xt, w_sb, dst, bias_off, ci, pair):
            h = [None, None]
            proj_qk_piece(xt, w_sb, dst, bias_off, ci, pair, range(NDC), h)

        def proj_v_piece(xt, ci, c, dc_range, pv_holder):
            """Part of one t-tile of the natural-layout V projection."""
            tt = ci * (SBLK // P) + c
            if dc_range[0] == 0:
                pv_holder[0] = psum.tile([P, 2, SBLK], FP32, tag="sc",
                                         name="pv", bufs=3)
            pv = pv_holder[0]
            for dc in dc_range:
                nc.tensor.matmul(pv[:, 0, 0:NV], lhsT=xt[:, dc, c * P:(c + 1) * P],
                                 rhs=wv_sb[:, dc, :], start=(dc == 0), stop=False)
            if dc_range[-1] == NDC - 1:
                nc.tensor.matmul(pv[:, 0, 0:NV], lhsT=brow[:, ONES_OFF:ONES_OFF + P],
                                 rhs=brow[:, BVE_OFF:BVE_OFF + NV],
                                 start=False, stop=True)
                nc.vector.tensor_copy(vw[:, tt, :], pv[:, 0, 0:NV])

        def proj_v_tt(xt, ci, c):
            h = [None]
            proj_v_piece(xt, ci, c, range(NDC), h)

        def attn_block(pair, b, fillers):
            """Per-t-tile pipeline: scores(k) -> exp(k) -> ctx(k-2).
            One 2-bank scores PSUM per step (hp0 | hp1), 3-deep rotation;
            exp is a single FD=1024 ACT instruction. `fillers` is a list of
            (slot, fn); fn is emitted when the step index reaches slot."""
            ct = [psum.tile([DV + 1, SBLK], FP32, tag=f"ct{hp}", name=f"ct{hp}")
                  for hp in range(2)]
            exs = {}
            for k in range(NTT + 2):
                if k < NTT:
                    sc = psum.tile([P, 2, SBLK], FP32, tag="sc", name="sc", bufs=3)
                    for hp in range(2):
                        lo, hi = hp * DK, (hp + 1) * DK
                        nc.tensor.matmul(
                            sc[:, hp, :],
                            lhsT=kwt[pair][lo:hi, k * P:(k + 1) * P],
                            rhs=qwt[pair][lo:hi, b * SBLK:(b + 1) * SBLK],
                            start=True, stop=True)
                    ex = expp.tile([P, 2, SBLK], BF16, tag="ex", name="ex", bufs=4)
                    nc.scalar.activation(ex[:], sc[:],
                                         mybir.ActivationFunctionType.Exp)
                    exs[k] = ex
                while fillers and fillers[0][0] <= k:
                    fillers.pop(0)[1]()
                kc = k - 2
                if kc >= 0:
                    ex = exs.pop(kc)
                    for hp in range(2):
                        hh = 2 * pair + hp
                        nc.tensor.matmul(
                            ct[hp][:], lhsT=vw[:, kc, hh * (DV + 1):(hh + 1) * (DV + 1)],
                            rhs=ex[:, hp, :],
                            start=(kc == 0), stop=(kc == NTT - 1))
            return ct

        def attn_norm_den(ct, hp, den_h):
            # DVE-only: pull the denominator row out of PSUM a step before
            # the PE broadcast matmul needs it (avoids PE queue parking).
            den_h[hp] = smallp.tile([1, SBLK], FP32R, tag="den", name="den")
            nc.vector.tensor_copy(den_h[hp][:], ct[hp][DV:DV + 1, :])

        def attn_norm_fin(pair, b, ct, hp, den_h):
            # ctx = ct[0:64] * (1 / ct[64]) row-broadcast; fp16 out
            rb = psum.tile([P, 2, SBLK], FP32, tag="sc", name="rb", bufs=3)
            nc.tensor.matmul(rb[0:DV, 0, :], lhsT=onesdv[:],
                             rhs=den_h[hp][:], start=True, stop=True)
            rcp = smallp.tile([DV, SBLK], FP32, tag="rcp")
            nc.vector.reciprocal_approx_fast(rcp[:], rb[0:DV, 0, :])
            nc.vector.tensor_mul(
                ctx_t[pair][hp * DV:(hp + 1) * DV, b * SBLK:(b + 1) * SBLK],
                ct[hp][0:DV, :], rcp[:])

        def attn_normalize(pair, b, ct, hp, den_h=None):
            if den_h is None:
                den_h = [None, None]
            attn_norm_den(ct, hp, den_h)
            attn_norm_fin(pair, b, ct, hp, den_h)

        ob_holder = [None]

        def out_proj_st(b, st):
            """One s-tile of the output projection: [128 s, 1024 D] via 4
            N=512 matmuls (fp16 moving operand caps at 512); the 4 s-tiles
            of a block stage into one SBUF tile DMA'd out as a single 1MB
            transfer on the gpsimd queue."""
            off = b * SBLK + st * P
            if st == 0:
                ob_holder[0] = outp.tile([P, 4, D], FP16, tag="ob", name="ob")
            po = psum.tile([P, 2, SBLK], FP32, tag="sc", name="po", bufs=3)
            for nh in range(2):
                for jc in range(2):
                    nc.tensor.matmul(po[:, nh, :],
                                     lhsT=ctx_t[jc][:, off:off + P],
                                     rhs=wo_sb[:, jc, nh * SBLK:(nh + 1) * SBLK],
                                     start=(jc == 0), stop=(jc == 1))
            nc.vector.tensor_copy(ob_holder[0][:, st, :],
                                  po[:].rearrange("p u q -> p (u q)"))
            if b == NBLK - 1:
                nc.gpsimd.dma_start(out_d[b][:, st * D:(st + 1) * D],
                                    ob_holder[0][:, st, :])
            elif st == 3:
                nc.gpsimd.dma_start(out_d[b],
                                    ob_holder[0][:].rearrange("p a b -> p (a b)"))

        # ---- emission schedule ----
        # Minimal prologue: K/Q chunk 0 land first and block 0 pair 0 starts
        # immediately; V chunk 0 feeds the (2-step-trailing) ctx matmuls.
        # Everything else — K/V chunks 1-3, Q chunks 1-3, out-proj,
        # normalize — drains into the per-step slack of the blocks as small
        # (<=4-matmul) filler units, slotted so each unit is emitted strictly
        # before its consumer but late enough that its DMA has landed (a
        # piece waiting on DMA at the PE queue head stalls everything).
        kt0 = load_chunk("kt", 0)
        nc.gpsimd.dma_start(wq_sb[:].rearrange("p a b -> p (a b)"), d["wq"])
        qt0 = load_chunk("qt", 0, nc.gpsimd)
        vt0 = load_chunk("vt", 0, nc.gpsimd)
        proj_qk(kt0, wk_sb, kwt, BK_OFF, 0, 0)
        proj_qk(kt0, wk_sb, kwt, BK_OFF, 0, 1)
        proj_qk(qt0, wq_sb, qwt, BQ_OFF, 0, 0)
        proj_qk(qt0, wq_sb, qwt, BQ_OFF, 0, 1)
        kts = {1: load_chunk("kt", 1)}
        vts = {0: vt0, 1: load_chunk("vt", 1, nc.gpsimd)}
        kts[2] = load_chunk("kt", 2)
        vts[2] = load_chunk("vt", 2, nc.gpsimd)
        nc.sync.dma_start(wo_sb[:].rearrange("p a b -> p (a b)"), d["wo"])
        qts = {0: qt0}

        def k_fillers(ci, pair, s0):
            """3 pieces: dc 0-2, 3-5, 6-7+bias+evict."""
            holder = [None, None]
            return [(s0 + j, lambda r=tuple(rr), h=holder, c=ci, p=pair:
                     proj_qk_piece(kts[c], wk_sb, kwt, BK_OFF, c, p, r, h))
                    for j, rr in enumerate(([0, 1, 2], [3, 4, 5], [6, 7]))]

        def q_fillers(ci, s0):
            out = []
            slot = s0
            for pair in range(2):
                holder = [None, None]
                for rr in ([0, 1, 2], [3, 4, 5], [6, 7]):
                    out.append((slot, lambda p=pair, r=tuple(rr), h=holder, c=ci:
                                proj_qk_piece(qts[c], wq_sb, qwt, BQ_OFF, c, p, r, h)))
                    slot += 1
            return out

        def v_fillers(ci, s0):
            """2 pieces per t-tile at slots (s0+c, s0+c+1): piece 2 lands one
            step before ctx(tt) consumes the tile (ctx trails by 2)."""
            out = []
            for c in range(4):
                holder = [None]
                for j, rr in enumerate(([0, 1, 2, 3], [4, 5, 6, 7])):
                    out.append((s0 + c + j,
                                lambda cc=c, r=tuple(rr), h=holder, ci_=ci:
                                proj_v_piece(vts[ci_], ci_, cc, r, h)))
            return out

        # b0p0: stream in K chunks 1-3 (pair 0) and all V chunks in slack.
        fill = sorted(
            v_fillers(0, 0)
            + k_fillers(1, 0, 1)
            + k_fillers(1, 1, 2)  # pair-1 c1 must fully consume kt1 before
                                  # vt3's DMA (slot 5) recycles its buffer
            + [(5, lambda: kts.__setitem__(3, load_chunk("kt", 3))),
               (6, lambda: vts.__setitem__(3, load_chunk("vt", 3, nc.gpsimd)))]
            + k_fillers(2, 0, 4)
            + v_fillers(1, 4)
            + k_fillers(3, 0, 8)
            + v_fillers(2, 8)
            + v_fillers(3, 12),
            key=lambda x: x[0])
        ct = attn_block(0, 0, fill)
        prev = (0, 0, ct)

        # remaining 7 pair-blocks
        for b in range(NBLK):
            for pair in range(2):
                if b == 0 and pair == 0:
                    continue
                fill = []
                pp, pb, pct = prev
                den_h = [None, None]
                fill.append((0, lambda c=pct, dh=den_h: attn_norm_den(c, 0, dh)))
                fill.append((0, lambda c=pct, dh=den_h: attn_norm_den(c, 1, dh)))
                fill.append((1, lambda p=pp, bb=pb, c=pct, dh=den_h:
                             attn_norm_fin(p, bb, c, 0, dh)))
                fill.append((2, lambda p=pp, bb=pb, c=pct, dh=den_h:
                             attn_norm_fin(p, bb, c, 1, dh)))
                if b == 0 and pair == 1:
                    # pair-1 K projections (chunk ci first needed by scores
                    # at step 4*ci, so pieces must sit at slots <= 4*ci-1)
                    fill += k_fillers(2, 1, 4)
                    fill += k_fillers(3, 1, 7)
                if pair == 1 and b < NBLK - 1:
                    # next block's Q chunk: DMA now, project late in block
                    qts[b + 1] = load_chunk("qt", b + 1)
                    fill += q_fillers(b + 1, 10)
                if pair == 0 and b > 0:
                    for st in range(4):
                        fill.append((4 + st * 3, lambda bb=b - 1, s=st:
                                     out_proj_st(bb, s)))
                fill.sort(key=lambda x: x[0])
                ct = attn_block(pair, b, fill)
                prev = (pair, b, ct)
        attn_normalize(1, NBLK - 1, ct, 0)
        attn_normalize(1, NBLK - 1, ct, 1)
        for st in range(4):
            out_proj_st(NBLK - 1, st)

    nc.compile()
    return nc


_NC_CACHE = None


def _get_nc():
    global _NC_CACHE
    if _NC_CACHE is None:
        _NC_CACHE = _build_nc()
    return _NC_CACHE


def _chunked(xT):
    """[D, S] -> [NBLK, P, NDC*SBLK]: chunk tile (p, dc, s) = xT[dc*128+p,
    ci*512+s], laid out so each partition's chunk row is contiguous."""
    x = xT.reshape(NDC, P, NBLK, SBLK).transpose(2, 1, 0, 3)
    return np.ascontiguousarray(x.reshape(NBLK, P, NDC * SBLK)).astype(np.float16)


def _wpack(w, cols):
    """[D, cols] -> [P, NDC*cols] with (p, dc, m) = w[dc*128+p, m]."""
    x = w.reshape(NDC, P, cols).transpose(1, 0, 2)
    return np.ascontiguousarray(x.reshape(P, NDC * cols)).astype(np.float16)


def kernel(Q, K, V, Wq, bq, Wk, bk, Wv, bv, Wo, bo, _trace=False, _trace_kwargs=None):
    nc = _get_nc()
    qt_h = [_chunked(np.asarray(Q[b]).T) for b in range(B)]
    kt_h = [_chunked(np.asarray(K[b]).T) for b in range(B)]
    vt_h = [_chunked(np.asarray(V[b]).T) for b in range(B)]
    onesdv = np.ones((1, DV), dtype=np.float32)

    in_maps = []
    for c in range(N_CORES):
        b, g = c % B, c // B
        hs = list(range(g * HPC, (g + 1) * HPC))
        wq_p = np.concatenate([Wq[h] for h in hs], axis=1)
        wk_p = np.concatenate([Wk[h] for h in hs], axis=1)
        wv_e = np.zeros((D, NV), dtype=np.float32)
        bv_e = np.zeros(NV, dtype=np.float32)
        for i, h in enumerate(hs):
            wv_e[:, i * (DV + 1):i * (DV + 1) + DV] = Wv[h] * SCALE
            bv_e[i * (DV + 1):i * (DV + 1) + DV] = bv[h] * SCALE
            bv_e[i * (DV + 1) + DV] = 1.0
        brow = np.zeros((1, BROW_W), dtype=np.float32)
        brow[0, ONES_OFF:ONES_OFF + SBLK] = 1.0
        brow[0, BQ_OFF:BQ_OFF + 2 * P] = np.concatenate([bq[h] for h in hs])
        brow[0, BK_OFF:BK_OFF + 2 * P] = np.concatenate([bk[h] for h in hs])
        brow[0, BVE_OFF:BVE_OFF + NV] = bv_e
        wo_g = np.asarray(Wo[g * HPC * DV:(g + 1) * HPC * DV])  # [256, 1024]
        wo_p = wo_g.reshape(2, P, D).transpose(1, 0, 2).reshape(P, 2 * D)
        in_maps.append({
            "qt": qt_h[b], "kt": kt_h[b], "vt": vt_h[b],
            "wq": _wpack(wq_p, 2 * P),
            "wk": _wpack(wk_p, 2 * P),
            "wv": _wpack(wv_e, NV),
            "wo": np.ascontiguousarray(wo_p).astype(np.float16),
            "brow": brow.astype(np.float16),
            "onesdv": onesdv,
        })

    kw = {}
    if _trace:
        kw = dict(trace=True, **(_trace_kwargs or {}))
    res = run_bass_kernel_spmd(nc, in_maps, core_ids=list(range(N_CORES)), **kw)

    out = np.zeros((B, S, D), dtype=np.float32)
    for c in range(N_CORES):
        o = np.asarray(res.results[c]["out"], dtype=np.float32)
        # [NBLK, P, 4*D]: row s = b*512 + st*128 + p
        o = o.reshape(NBLK, P, 4, D).transpose(0, 2, 1, 3).reshape(S, D)
        out[c % B] += o
    out += bo[None, None, :]
    if _trace:
        return out, res
    return out
